# revision 21
# baseline (speedup 1.0000x reference)
"""Trainium2 Bass kernel for BSQ (binary spherical quantization) codebook forward.

Math: out = sign(x @ W_enc.T + b_enc) @ W_dec.T + b_dec
(The L2-normalize in the reference is a forward no-op: dividing by a positive
norm never changes the sign, and the eps-clamped zero-vector case produces
sign(0)=+1 either way.)

Strategy (pure data parallel over 8 NeuronCores, 8192 tokens each):
- x rides to the device as an fp16 hi part plus a scaled fp8-e4m3 residual
  (xl8 = clip((x - fp16(x)) * 2^13), shipped as uint8 bits since the PJRT
  path rejects fp8 dtypes), all fully transposed on the host into chunk-major
  feature-on-partition layout so the device sees plain DMAs only at full HBM
  rate: 3 B/elem input instead of 4. Input loads ride the SP HWDGE ring,
  output stores the ACT HWDGE ring, weights the gpsimd SWDGE path.
- Blocks of [512, 1024 x 7, 512] tokens: the small edge blocks shrink the
  pipeline fill/drain, the 1 MB-per-stream block DMAs keep descriptors big.
- mm1 hi: z.T[17,512] per 512-token subtile accumulated in PSUM from 8 fp16
  matmuls (xh@Wh + xh@Wl per 128-wide K-chunk), subtiles column-packed into
  PE strips (tile_position=(0,32s)). Row 16 of z is forced to 0 by a zero
  weight column.
- mm1 residual: zr.T = xl8 @ (Wh*2^6 as fp8) via plain fp8 matmuls (PE has
  headroom; DoubleRow trips ISA restrictions) into a separate accumulator.
- sign: two full-tile DVE ops per block: tmp = -zr*2^-19 + (-b_enc) then
  q = (z >= tmp) via scalar_tensor_tensor; threshold row 32s+16 is -1 so q
  gets its "+1" bias row for free (0 >= -1 -> 1.0).
- mm2: out[128,512] = q_aug[17,:].T @ [2*W_dec.T ; b_dec - W_dec.sum(1)],
  one matmul per 128 tokens, g-outer interleave so consecutive matmuls sit
  in different PE row bands (tile_position=(32s,0)) and weight loads overlap
  streaming.
- Epilogue: PSUM->SBUF copies split DVE/ScalarE; output DRAM is laid out as
  [TOK//SUB, 128, 4*512] so each 1 MB store has 8 KB-contiguous partition
  lines (big write descriptors); the host reassembles token order.
"""

import numpy as np
import ml_dtypes

import concourse.bacc as bacc
import concourse.mybir as mybir
from concourse import tile
from concourse.bass_utils import run_bass_kernel_spmd

NCORES = 8
B, H, W_, D = 64, 32, 32, 512
C = 16            # codebook bits
CA = C + 1        # + the constant-one row for the decoder bias
P = 128           # partitions
NCH = D // P      # 4 K-chunks for the encoder contraction
TOK = (B // NCORES) * H * W_   # 8192 tokens per core
SUB = 512         # tokens per z subtile (one PSUM accumulation group)
BLOCKS = (512, 1024, 1024, 1024, 1024, 1024, 1024, 1024, 512)   # token blocks (sum = TOK)
NSLAB = TOK // SUB        # 16 output slabs of [128, 4*512]
W1C = 2 * NCH * CA        # 136 w1 columns (8 fp16 lhsT tiles of [128, 17])
XS = 2.0 ** 13            # residual pre-scale
WS = 2.0 ** 6             # fp8 weight pre-scale
RS = -1.0 / (XS * WS)     # combine scale (negated: tmp = -zr*2^-19 + negb)

F8 = ml_dtypes.float8_e4m3

_CACHE = {}


def _build_nc():
    f16, f32 = mybir.dt.float16, mybir.dt.float32
    f8, u8 = mybir.dt.float8e4, mybir.dt.uint8
    nc = bacc.Bacc(
        "TRN2",
        target_bir_lowering=False,
        debug=False,
        enable_asserts=False,
        num_devices=NCORES,
    )
    xh = nc.dram_tensor("xh", [P, NCH * TOK], f16, kind="ExternalInput").ap()
    xl8 = nc.dram_tensor("xl8", [P, NCH * TOK], u8, kind="ExternalInput").ap()
    w1 = nc.dram_tensor("w1", [P, W1C], f16, kind="ExternalInput").ap()
    w8 = nc.dram_tensor("w8", [P, NCH * CA], u8, kind="ExternalInput").ap()
    w2 = nc.dram_tensor("w2", [P, D], f16, kind="ExternalInput").ap()
    negb = nc.dram_tensor("negb", [P, 1], f32, kind="ExternalInput").ap()
    out = nc.dram_tensor("out", [NSLAB, P, NCH * SUB], f16, kind="ExternalOutput").ap()

    with tile.TileContext(nc) as tc:
        with (
            tc.tile_pool(name="consts", bufs=1) as cpool,
            tc.tile_pool(name="xt", bufs=9) as xpool,
            tc.tile_pool(name="xr", bufs=9) as xrpool,
            tc.tile_pool(name="q", bufs=2) as qpool,
            tc.tile_pool(name="tmp", bufs=2) as tpool,
            tc.tile_pool(name="osb", bufs=8) as opool,
            tc.tile_pool(name="zps", bufs=2, space="PSUM") as zpool,
            tc.tile_pool(name="zrps", bufs=2, space="PSUM") as zrpool,
            tc.tile_pool(name="ops", bufs=4, space="PSUM") as opspool,
        ):
            # PE warmup: the HAM clock gate holds the PE at 1.2 GHz until
            # ~3.4 us of sustained activity. Burn dummy matmuls on a memset
            # tile during the preamble/first-DMA window so real mm1 starts
            # at 2.4 GHz. The tensor_copy read releases the borrowed zr buf.
            dum_in = cpool.tile([P, SUB], f16)
            nc.vector.memset(dum_in[:], 0.0)
            zdum = zrpool.tile([P, SUB], f32, tag="zr", name="zdum")
            for i in range(9):
                nc.tensor.matmul(
                    zdum[0:CA, :],
                    dum_in[:, 0:CA],
                    dum_in[:],
                    start=True,
                    stop=True,
                    skip_group_check=True,
                )
            dum_out = cpool.tile([P, 4], f32)
            nc.vector.tensor_copy(out=dum_out[0:CA, :], in_=zdum[0:CA, 0:4])

            # Weights ride the SWDGE (gpsimd) path so both HWDGE rings start
            # streaming bulk data at t=0.
            w1_sb = cpool.tile([P, W1C], f16)
            nc.gpsimd.dma_start(out=w1_sb[:], in_=w1)
            w8_sb = cpool.tile([P, NCH * CA], u8)
            nc.gpsimd.dma_start(out=w8_sb[:], in_=w8)
            w2_sb = cpool.tile([P, D], f16)
            nc.gpsimd.dma_start(out=w2_sb[:], in_=w2)
            negb_sb = cpool.tile([P, 1], f32)
            nc.gpsimd.dma_start(out=negb_sb[:], in_=negb)

            # Streaming transposed x, block-major so block b unlocks early.
            nblk = len(BLOCKS)
            starts = [sum(BLOCKS[:i]) for i in range(nblk)]
            xh_b = [xpool.tile([P, NCH * BLOCKS[b]], f16, tag="xt", name=f"xh{b}")
                    for b in range(nblk)]
            xr_b = [xrpool.tile([P, NCH * BLOCKS[b]], u8, tag="xr", name=f"xr{b}")
                    for b in range(nblk)]
            for b in range(nblk):
                st, nt = starts[b], BLOCKS[b]
                nc.sync.dma_start(
                    out=xh_b[b][:], in_=xh[:, NCH * st:NCH * (st + nt)]
                )
                nc.sync.dma_start(
                    out=xr_b[b][:], in_=xl8[:, NCH * st:NCH * (st + nt)]
                )

            for b in range(nblk):
                st, nt = starts[b], BLOCKS[b]
                nsub = nt // SUB
                z_ps = zpool.tile([P, SUB], f32, tag="z", name=f"z{b}")
                zr_ps = zrpool.tile([P, SUB], f32, tag="zr", name=f"zr{b}")
                # mm1 hi: two fp16 products per chunk, column-packed across
                # the block's subtiles.
                for ci in range(NCH):
                    for p in range(2):
                        wofs = ((p * NCH) + ci) * CA
                        for s in range(nsub):
                            nc.tensor.matmul(
                                z_ps[32 * s:32 * s + CA, :],
                                w1_sb[:, wofs:wofs + CA],
                                xh_b[b][:, ci * nt + s * SUB:ci * nt + (s + 1) * SUB],
                                start=(ci == 0 and p == 0),
                                stop=(ci == NCH - 1 and p == 1),
                                tile_position=(0, 32 * s),
                                skip_group_check=True,
                            )
                # mm1 residual: plain fp8 matmuls, one per chunk.
                for ci in range(NCH):
                    for s in range(nsub):
                        nc.tensor.matmul(
                            zr_ps[32 * s:32 * s + CA, :],
                            w8_sb[:, ci * CA:(ci + 1) * CA].bitcast(f8),
                            xr_b[b][:, ci * nt + s * SUB:ci * nt + (s + 1) * SUB].bitcast(f8),
                            start=(ci == 0),
                            stop=(ci == NCH - 1),
                            tile_position=(0, 32 * s),
                            skip_group_check=True,
                        )
                # sign combine: tmp = -zr*2^-19 + (-b); q = (z >= tmp).
                nrow = 32 * (nsub - 1) + CA
                tmp_sb = tpool.tile([P, SUB], f32, tag="tmp", name=f"tmp{b}")
                nc.vector.tensor_scalar(
                    out=tmp_sb[:nrow, :],
                    in0=zr_ps[:nrow, :],
                    scalar1=RS,
                    scalar2=negb_sb[:nrow, :],
                    op0=mybir.AluOpType.mult,
                    op1=mybir.AluOpType.add,
                )
                q_sb = qpool.tile([P, SUB], f16, tag="q", name=f"q{b}")
                nc.vector.scalar_tensor_tensor(
                    out=q_sb[:nrow, :],
                    in0=z_ps[:nrow, :],
                    scalar=1.0,
                    in1=tmp_sb[:nrow, :],
                    op0=mybir.AluOpType.mult,
                    op1=mybir.AluOpType.is_ge,
                )
                # mm2 + epilogue: g-outer so consecutive matmuls sit in
                # different PE row bands and LDWEIGHTS overlaps streaming.
                o_sbs = [
                    opool.tile([P, NCH * SUB], f16, tag="osb", name=f"osb{b}_{s}")
                    for s in range(nsub)
                ]
                for g in range(NCH):
                    for s in range(nsub):
                        o_ps = opspool.tile([P, D], f32, tag="ops", name=f"ops{b}_{g}_{s}")
                        nc.tensor.matmul(
                            o_ps[:],
                            q_sb[32 * s:32 * s + CA, g * P:(g + 1) * P],
                            w2_sb[32 * s:32 * s + CA, :],
                            start=True,
                            stop=True,
                            tile_position=(32 * s, 0),
                            skip_group_check=True,
                        )
                        if (g * nsub + s) % 2 == 0:
                            nc.vector.tensor_copy(out=o_sbs[s][:, g * D:(g + 1) * D], in_=o_ps[:])
                        else:
                            nc.scalar.copy(out=o_sbs[s][:, g * D:(g + 1) * D], in_=o_ps[:])
                for s in range(nsub):
                    slab = st // SUB + s
                    nc.gpsimd.dma_start(out=out[slab], in_=o_sbs[s][:])
    nc.compile()
    return nc


def _get_nc():
    if "nc" not in _CACHE:
        _CACHE["nc"] = _build_nc()
    return _CACHE["nc"]


def _prep_weights(W_enc, b_enc, W_dec, b_dec):
    f16, f32 = np.float16, np.float32
    WT = np.ascontiguousarray(W_enc.T.astype(f32))            # [512, 16]
    Wh = WT.astype(f16)
    Wl = (WT - Wh.astype(f32)).astype(f16)
    # 8 fp16 lhsT tiles of [128, 17], chunk-major products, col 16 = 0
    w1 = np.zeros((P, W1C), f16)
    for p in range(2):
        src_w = [Wh, Wl][p]
        for c in range(NCH):
            ofs = (p * NCH + c) * CA
            w1[:, ofs:ofs + C] = src_w[c * P:(c + 1) * P, :]
    # fp8 residual weights, chunk-major [128, NCH*17], pre-scaled by 2^6
    w8 = np.zeros((P, NCH * CA), F8)
    for c in range(NCH):
        w8[:, c * CA:c * CA + C] = (Wh[c * P:(c + 1) * P, :].astype(f32) * WS).astype(F8)

    # w2: replica of [2*W_dec.T ; bias_row] in each 32-row band.
    w2 = np.zeros((P, D), f16)
    band = np.concatenate(
        [2.0 * W_dec.T.astype(f32),
         (b_dec.astype(f32) - W_dec.astype(f32).sum(axis=1)).reshape(1, D)],
        axis=0,
    ).astype(f16)                                             # [17, 512]
    negb = np.full((P, 1), -1.0, f32)
    for s in range(NCH):
        w2[32 * s:32 * s + CA, :] = band
        negb[32 * s:32 * s + C, 0] = -b_enc.astype(f32)
        # row 32s+16 stays -1.0: z row is 0, 0 >= -1 -> q row = 1.0 (bias row)
    return w1, w8.view(np.uint8), w2, negb


def _prep_x_shard(x_flat_shard):
    """[8192, 512] fp32 -> xh [4, 128, 8192] fp16 (chunk-major, transposed)
    and xl8 [128, 4*8192] scaled fp8 residual bits (block-chunk-major)."""
    f16, f32 = np.float16, np.float32
    xh = x_flat_shard.astype(f16)
    xl = (x_flat_shard - xh.astype(f32)).astype(f32)
    xl8 = np.clip(xl * XS, -240.0, 240.0).astype(F8)
    xh = xh.reshape(TOK, NCH, P).transpose(2, 1, 0)           # [P, NCH, TOK]
    xl8 = xl8.reshape(TOK, NCH, P).transpose(2, 1, 0)         # [P, NCH, TOK]
    hcols, lcols = [], []
    st = 0
    for nt in BLOCKS:
        hcols.append(xh[:, :, st:st + nt].reshape(P, NCH * nt))
        lcols.append(xl8[:, :, st:st + nt].reshape(P, NCH * nt))
        st += nt
    xh = np.ascontiguousarray(np.concatenate(hcols, axis=1))
    xl8 = np.ascontiguousarray(np.concatenate(lcols, axis=1)).view(np.uint8)
    return xh, xl8


def kernel(x, W_enc, b_enc, W_dec, b_dec, _trace=False, _trace_kwargs=None):
    x = np.asarray(x, dtype=np.float32)
    w1, w8, w2, negb = _prep_weights(
        np.asarray(W_enc), np.asarray(b_enc), np.asarray(W_dec), np.asarray(b_dec)
    )
    xf = x.reshape(NCORES, TOK, D)
    in_maps = []
    for s in range(NCORES):
        xh, xl8 = _prep_x_shard(xf[s])
        in_maps.append(dict(xh=xh, xl8=xl8, w1=w1, w8=w8, w2=w2, negb=negb))
    nc = _get_nc()
    res = run_bass_kernel_spmd(
        nc,
        in_maps,
        core_ids=list(range(NCORES)),
        trace=_trace,
        **(_trace_kwargs or {}),
    )
    out = np.concatenate(
        [
            res.results[s]["out"]                      # [16, 128, 2048] f16
            .astype(np.float32)
            .reshape(NSLAB, P, NCH, D)
            .transpose(0, 2, 1, 3)                     # (slab, g, p, d)
            .reshape(1, TOK, D)
            for s in range(NCORES)
        ],
        axis=0,
    ).reshape(B, H, W_, D)
    _CACHE["last_results"] = res
    return out


# revision 22
# speedup vs baseline: 1.1470x; 1.1470x over previous
"""Trainium2 Bass kernel for BSQ (binary spherical quantization) codebook forward.

Math: out = sign(x @ W_enc.T + b_enc) @ W_dec.T + b_dec
(The L2-normalize in the reference is a forward no-op: dividing by a positive
norm never changes the sign, and the eps-clamped zero-vector case produces
sign(0)=+1 either way.)

Strategy (pure data parallel over 8 NeuronCores, 8192 tokens each):
- x rides to the device as an fp16 hi part plus a scaled fp8-e4m3 residual
  (xl8 = clip((x - fp16(x)) * 2^13), shipped as uint8 bits since the PJRT
  path rejects fp8 dtypes), all fully transposed on the host into chunk-major
  feature-on-partition layout so the device sees plain DMAs only at full HBM
  rate: 3 B/elem input instead of 4. Input loads ride the SP HWDGE ring,
  output stores the ACT HWDGE ring, weights the gpsimd SWDGE path.
- Blocks of [512, 1024 x 7, 512] tokens: the small edge blocks shrink the
  pipeline fill/drain, the 1 MB-per-stream block DMAs keep descriptors big.
- mm1 hi: z.T[17,512] per 512-token subtile accumulated in PSUM from 8 fp16
  matmuls (xh@Wh + xh@Wl per 128-wide K-chunk), subtiles column-packed into
  PE strips (tile_position=(0,32s)). Row 16 of z is forced to 0 by a zero
  weight column.
- mm1 residual: zr.T = xl8 @ (Wh*2^6 as fp8) via plain fp8 matmuls (PE has
  headroom; DoubleRow trips ISA restrictions) into a separate accumulator.
- sign: two full-tile DVE ops per block: tmp = -zr*2^-19 + (-b_enc) then
  q = (z >= tmp) via scalar_tensor_tensor; threshold row 32s+16 is -1 so q
  gets its "+1" bias row for free (0 >= -1 -> 1.0).
- mm2: out[128,512] = q_aug[17,:].T @ [2*W_dec.T ; b_dec - W_dec.sum(1)],
  one matmul per 128 tokens, g-outer interleave so consecutive matmuls sit
  in different PE row bands (tile_position=(32s,0)) and weight loads overlap
  streaming.
- Epilogue: PSUM->SBUF copies split DVE/ScalarE; output DRAM is laid out as
  [TOK//SUB, 128, 4*512] so each 1 MB store has 8 KB-contiguous partition
  lines (big write descriptors); the host reassembles token order.
"""

import numpy as np
import ml_dtypes

import concourse.bacc as bacc
import concourse.mybir as mybir
from concourse import tile
from concourse.bass_utils import run_bass_kernel_spmd

NCORES = 8
B, H, W_, D = 64, 32, 32, 512
C = 16            # codebook bits
CA = C + 1        # + the constant-one row for the decoder bias
P = 128           # partitions
NCH = D // P      # 4 K-chunks for the encoder contraction
TOK = (B // NCORES) * H * W_   # 8192 tokens per core
SUB = 512         # tokens per z subtile (one PSUM accumulation group)
BLOCKS = (512, 1024, 1024, 1024, 1024, 1024, 1024, 1024, 512)   # token blocks (sum = TOK)
NSLAB = TOK // SUB        # 16 output slabs of [128, 4*512]
W1C = 2 * NCH * CA        # 136 w1 columns (8 fp16 lhsT tiles of [128, 17])
XS = 2.0 ** 13            # residual pre-scale
WS = 2.0 ** 6             # fp8 weight pre-scale
RS = -1.0 / (XS * WS)     # combine scale (negated: tmp = -zr*2^-19 + negb)

F8 = ml_dtypes.float8_e4m3

_CACHE = {}


def _build_nc():
    f16, f32 = mybir.dt.float16, mybir.dt.float32
    f8, u8 = mybir.dt.float8e4, mybir.dt.uint8
    nc = bacc.Bacc(
        "TRN2",
        target_bir_lowering=False,
        debug=False,
        enable_asserts=False,
        num_devices=NCORES,
    )
    xh = nc.dram_tensor("xh", [P, NCH * TOK], f16, kind="ExternalInput").ap()
    xl8 = nc.dram_tensor("xl8", [P, NCH * TOK], u8, kind="ExternalInput").ap()
    w1 = nc.dram_tensor("w1", [P, W1C], f16, kind="ExternalInput").ap()
    w8 = nc.dram_tensor("w8", [P, NCH * CA], u8, kind="ExternalInput").ap()
    w2 = nc.dram_tensor("w2", [P, D], f16, kind="ExternalInput").ap()
    negb = nc.dram_tensor("negb", [P, 1], f32, kind="ExternalInput").ap()
    out = nc.dram_tensor("out", [NSLAB, P, NCH * SUB], f16, kind="ExternalOutput").ap()

    with tile.TileContext(nc) as tc:
        with (
            tc.tile_pool(name="consts", bufs=1) as cpool,
            tc.tile_pool(name="xt", bufs=9) as xpool,
            tc.tile_pool(name="xr", bufs=9) as xrpool,
            tc.tile_pool(name="q", bufs=2) as qpool,
            tc.tile_pool(name="tmp", bufs=2) as tpool,
            tc.tile_pool(name="osb", bufs=8) as opool,
            tc.tile_pool(name="zps", bufs=2, space="PSUM") as zpool,
            tc.tile_pool(name="zrps", bufs=2, space="PSUM") as zrpool,
            tc.tile_pool(name="ops", bufs=4, space="PSUM") as opspool,
        ):
            # Weights ride the SWDGE (gpsimd) path so both HWDGE rings start
            # streaming bulk data at t=0.
            w1_sb = cpool.tile([P, W1C], f16)
            nc.gpsimd.dma_start(out=w1_sb[:], in_=w1)
            w8_sb = cpool.tile([P, NCH * CA], u8)
            nc.gpsimd.dma_start(out=w8_sb[:], in_=w8)
            w2_sb = cpool.tile([P, D], f16)
            nc.gpsimd.dma_start(out=w2_sb[:], in_=w2)
            negb_sb = cpool.tile([P, 1], f32)
            nc.gpsimd.dma_start(out=negb_sb[:], in_=negb)

            # Streaming transposed x, block-major so block b unlocks early.
            nblk = len(BLOCKS)
            starts = [sum(BLOCKS[:i]) for i in range(nblk)]
            xh_b = [xpool.tile([P, NCH * BLOCKS[b]], f16, tag="xt", name=f"xh{b}")
                    for b in range(nblk)]
            xr_b = [xrpool.tile([P, NCH * BLOCKS[b]], u8, tag="xr", name=f"xr{b}")
                    for b in range(nblk)]
            for b in range(nblk):
                st, nt = starts[b], BLOCKS[b]
                nc.sync.dma_start(
                    out=xh_b[b][:], in_=xh[:, NCH * st:NCH * (st + nt)]
                )
                nc.sync.dma_start(
                    out=xr_b[b][:], in_=xl8[:, NCH * st:NCH * (st + nt)]
                )

            for b in range(nblk):
                st, nt = starts[b], BLOCKS[b]
                nsub = nt // SUB
                z_ps = zpool.tile([P, SUB], f32, tag="z", name=f"z{b}")
                zr_ps = zrpool.tile([P, SUB], f32, tag="zr", name=f"zr{b}")
                # mm1 hi: two fp16 products per chunk, column-packed across
                # the block's subtiles.
                for ci in range(NCH):
                    for p in range(2):
                        wofs = ((p * NCH) + ci) * CA
                        for s in range(nsub):
                            nc.tensor.matmul(
                                z_ps[32 * s:32 * s + CA, :],
                                w1_sb[:, wofs:wofs + CA],
                                xh_b[b][:, ci * nt + s * SUB:ci * nt + (s + 1) * SUB],
                                start=(ci == 0 and p == 0),
                                stop=(ci == NCH - 1 and p == 1),
                                tile_position=(0, 32 * s),
                                skip_group_check=True,
                            )
                # mm1 residual: plain fp8 matmuls, one per chunk.
                for ci in range(NCH):
                    for s in range(nsub):
                        nc.tensor.matmul(
                            zr_ps[32 * s:32 * s + CA, :],
                            w8_sb[:, ci * CA:(ci + 1) * CA].bitcast(f8),
                            xr_b[b][:, ci * nt + s * SUB:ci * nt + (s + 1) * SUB].bitcast(f8),
                            start=(ci == 0),
                            stop=(ci == NCH - 1),
                            tile_position=(0, 32 * s),
                            skip_group_check=True,
                        )
                # sign combine: tmp = -zr*2^-19 + (-b); q = (z >= tmp).
                nrow = 32 * (nsub - 1) + CA
                tmp_sb = tpool.tile([P, SUB], f32, tag="tmp", name=f"tmp{b}")
                nc.vector.tensor_scalar(
                    out=tmp_sb[:nrow, :],
                    in0=zr_ps[:nrow, :],
                    scalar1=RS,
                    scalar2=negb_sb[:nrow, :],
                    op0=mybir.AluOpType.mult,
                    op1=mybir.AluOpType.add,
                )
                q_sb = qpool.tile([P, SUB], f16, tag="q", name=f"q{b}")
                nc.vector.scalar_tensor_tensor(
                    out=q_sb[:nrow, :],
                    in0=z_ps[:nrow, :],
                    scalar=1.0,
                    in1=tmp_sb[:nrow, :],
                    op0=mybir.AluOpType.mult,
                    op1=mybir.AluOpType.is_ge,
                )
                # mm2 + epilogue: g-outer so consecutive matmuls sit in
                # different PE row bands and LDWEIGHTS overlaps streaming.
                o_sbs = [
                    opool.tile([P, NCH * SUB], f16, tag="osb", name=f"osb{b}_{s}")
                    for s in range(nsub)
                ]
                for g in range(NCH):
                    for s in range(nsub):
                        o_ps = opspool.tile([P, D], f32, tag="ops", name=f"ops{b}_{g}_{s}")
                        nc.tensor.matmul(
                            o_ps[:],
                            q_sb[32 * s:32 * s + CA, g * P:(g + 1) * P],
                            w2_sb[32 * s:32 * s + CA, :],
                            start=True,
                            stop=True,
                            tile_position=(32 * s, 0),
                            skip_group_check=True,
                        )
                        if (g * nsub + s) % 2 == 0:
                            nc.vector.tensor_copy(out=o_sbs[s][:, g * D:(g + 1) * D], in_=o_ps[:])
                        else:
                            nc.scalar.copy(out=o_sbs[s][:, g * D:(g + 1) * D], in_=o_ps[:])
                for s in range(nsub):
                    slab = st // SUB + s
                    nc.gpsimd.dma_start(out=out[slab], in_=o_sbs[s][:])
    nc.compile()
    return nc


def _get_nc():
    if "nc" not in _CACHE:
        _CACHE["nc"] = _build_nc()
    return _CACHE["nc"]


def _prep_weights(W_enc, b_enc, W_dec, b_dec):
    f16, f32 = np.float16, np.float32
    WT = np.ascontiguousarray(W_enc.T.astype(f32))            # [512, 16]
    Wh = WT.astype(f16)
    Wl = (WT - Wh.astype(f32)).astype(f16)
    # 8 fp16 lhsT tiles of [128, 17], chunk-major products, col 16 = 0
    w1 = np.zeros((P, W1C), f16)
    for p in range(2):
        src_w = [Wh, Wl][p]
        for c in range(NCH):
            ofs = (p * NCH + c) * CA
            w1[:, ofs:ofs + C] = src_w[c * P:(c + 1) * P, :]
    # fp8 residual weights, chunk-major [128, NCH*17], pre-scaled by 2^6
    w8 = np.zeros((P, NCH * CA), F8)
    for c in range(NCH):
        w8[:, c * CA:c * CA + C] = (Wh[c * P:(c + 1) * P, :].astype(f32) * WS).astype(F8)

    # w2: replica of [2*W_dec.T ; bias_row] in each 32-row band.
    w2 = np.zeros((P, D), f16)
    band = np.concatenate(
        [2.0 * W_dec.T.astype(f32),
         (b_dec.astype(f32) - W_dec.astype(f32).sum(axis=1)).reshape(1, D)],
        axis=0,
    ).astype(f16)                                             # [17, 512]
    negb = np.full((P, 1), -1.0, f32)
    for s in range(NCH):
        w2[32 * s:32 * s + CA, :] = band
        negb[32 * s:32 * s + C, 0] = -b_enc.astype(f32)
        # row 32s+16 stays -1.0: z row is 0, 0 >= -1 -> q row = 1.0 (bias row)
    return w1, w8.view(np.uint8), w2, negb


def _prep_x_shard(x_flat_shard):
    """[8192, 512] fp32 -> xh [4, 128, 8192] fp16 (chunk-major, transposed)
    and xl8 [128, 4*8192] scaled fp8 residual bits (block-chunk-major)."""
    f16, f32 = np.float16, np.float32
    xh = x_flat_shard.astype(f16)
    xl = (x_flat_shard - xh.astype(f32)).astype(f32)
    xl8 = np.clip(xl * XS, -240.0, 240.0).astype(F8)
    xh = xh.reshape(TOK, NCH, P).transpose(2, 1, 0)           # [P, NCH, TOK]
    xl8 = xl8.reshape(TOK, NCH, P).transpose(2, 1, 0)         # [P, NCH, TOK]
    hcols, lcols = [], []
    st = 0
    for nt in BLOCKS:
        hcols.append(xh[:, :, st:st + nt].reshape(P, NCH * nt))
        lcols.append(xl8[:, :, st:st + nt].reshape(P, NCH * nt))
        st += nt
    xh = np.ascontiguousarray(np.concatenate(hcols, axis=1))
    xl8 = np.ascontiguousarray(np.concatenate(lcols, axis=1)).view(np.uint8)
    return xh, xl8


def kernel(x, W_enc, b_enc, W_dec, b_dec, _trace=False, _trace_kwargs=None):
    x = np.asarray(x, dtype=np.float32)
    w1, w8, w2, negb = _prep_weights(
        np.asarray(W_enc), np.asarray(b_enc), np.asarray(W_dec), np.asarray(b_dec)
    )
    xf = x.reshape(NCORES, TOK, D)
    in_maps = []
    for s in range(NCORES):
        xh, xl8 = _prep_x_shard(xf[s])
        in_maps.append(dict(xh=xh, xl8=xl8, w1=w1, w8=w8, w2=w2, negb=negb))
    nc = _get_nc()
    res = run_bass_kernel_spmd(
        nc,
        in_maps,
        core_ids=list(range(NCORES)),
        trace=_trace,
        **(_trace_kwargs or {}),
    )
    out = np.concatenate(
        [
            res.results[s]["out"]                      # [16, 128, 2048] f16
            .astype(np.float32)
            .reshape(NSLAB, P, NCH, D)
            .transpose(0, 2, 1, 3)                     # (slab, g, p, d)
            .reshape(1, TOK, D)
            for s in range(NCORES)
        ],
        axis=0,
    ).reshape(B, H, W_, D)
    _CACHE["last_results"] = res
    return out
